# revision 12
# baseline (speedup 1.0000x reference)
"""Trainium2 Bass kernel for nn_MoELayer (moe_routing).

Reference computation (B=8192 tokens, d=1024, E=8 experts, top-k=2):
    gating  = softmax(x @ gate_w + gate_b)                    # [B, E]
    mask    = top-2 one-hot scatter of gating                 # [B, E]
    blockm  = mask.reshape(B//d, d, E).max(axis=1)            # per 1024-row block
    out     = sum_e gating[:, e] * blockm[block(b), e] * (x @ W[:, e*d:(e+1)*d])

Key structural facts exploited here:
  * The combine uses the FULL softmax weights over all experts; the top-2
    mask enters only through the per-1024-row-block max. So the compute is
    dense: out = sum_e (gating*blockmask) .* (x @ W_e).
  * Both the gating and the block mask for a 1024-row block depend only on
    that block's rows.

Sharding: data-parallel over the 8 row blocks of 1024 tokens (one per
NeuronCore). Each core computes its whole output block locally; there is
no cross-core communication. W is streamed (replicated) to every core.

Per-core kernel:
  * x^T block (d on partitions) is DMA'd in fp32; gating logits are
    computed on the PE in fp32 (exact), softmax on ACT/DVE in fp32.
  * top-2 mask via (v >= second_max) per row; block mask via a
    ones-vector matmul partition-reduction; broadcast back with a K=1
    matmul.
  * The 8 [1024x1024x1024] expert matmuls run in bf16 (fp32 PSUM
    accumulation).  Per-expert combine: ACT copies PSUM->SBUF scaled by
    the per-token gating column (per-partition scale), DVE accumulates.
"""

import numpy as np

P = 128          # partitions
D = 1024         # d_model
E = 8            # experts
TOK = 1024       # tokens per core (row block)
KT = D // P      # contraction tiles
MT = TOK // P    # token tiles
NH = 512         # psum half-width (one fp32 bank)
N_CORES = 8
WARMUP_MMS = 40  # PE warm-up matmuls issued while input DMAs are in flight


def _build_nc():
    import concourse.bacc as bacc
    import concourse.mybir as mybir
    import concourse.tile as tile

    f32 = mybir.dt.float32
    bf16 = mybir.dt.bfloat16
    AX = mybir.AxisListType
    OP = mybir.AluOpType
    AF = mybir.ActivationFunctionType

    # Bacc (not raw Bass): its compile() pass splits excess per-instruction
    # semaphore waits into standalone event-semaphore instructions and moves
    # matmul waits onto LDWEIGHTS — required for walrus codegen's per-
    # instruction sync-wait limits.
    nc = bacc.Bacc(None, target_bir_lowering=False, debug=False)
    xT_d = nc.dram_tensor("xT", [D, TOK], f32, kind="ExternalInput")
    w_d = nc.dram_tensor("w", [D, E * D], f32, kind="ExternalInput")
    gw_d = nc.dram_tensor("gate_w", [D, E], f32, kind="ExternalInput")
    gb_d = nc.dram_tensor("gate_b", [1, E], f32, kind="ExternalInput")
    out_d = nc.dram_tensor("out", [TOK, D], f32, kind="ExternalOutput")

    xT_r = xT_d.rearrange("(k p) t -> k p t", p=P)
    w_r = w_d.rearrange("(k p) (e f) -> k p e f", p=P, f=D)
    gw_r = gw_d.rearrange("(k p) e -> p k e", p=P)
    out_r = out_d.rearrange("(m p) f -> m p f", p=P)

    with tile.TileContext(nc) as tc:
        with (
            tc.tile_pool(name="persist", bufs=1) as persist,
            tc.tile_pool(name="gstat", bufs=2) as p_gs,
            tc.tile_pool(name="wf", bufs=8) as p_wf,
            tc.tile_pool(name="wb", bufs=2 * KT) as p_wb,
            tc.tile_pool(name="tmp", bufs=6) as p_tmp,
            tc.tile_pool(name="ps_gate", bufs=2, space="PSUM") as ps_gate,
            tc.tile_pool(name="ps_cnt", bufs=1, space="PSUM") as ps_cnt,
            tc.tile_pool(name="ps_mm", bufs=4, space="PSUM") as ps_mm,
        ):
            ones_col = persist.tile([P, 1], f32, tag="ones_col")
            nc.vector.memset(ones_col[:], 1.0)
            ones_row = persist.tile([1, P], f32, tag="ones_row")
            nc.vector.memset(ones_row[:], 1.0)
            # gate_b arrives by DMA; route it through a DVE copy so the PE
            # bias matmul waits on the DVE semaphore, not a DMA queue (the
            # LDWEIGHTS wait encoding can't take two DMA-queue waits).
            gb_in = persist.tile([1, E], f32, tag="gb_in")
            nc.sync.dma_start(gb_in[:], gb_d[:])
            gb_bf = persist.tile([1, E], bf16, tag="gb_bf")
            nc.vector.tensor_copy(gb_bf[:], gb_in[:])
            ones_row_bf = persist.tile([1, P], bf16, tag="ones_row_bf")
            nc.vector.memset(ones_row_bf[:], 1.0)
            bmb = persist.tile([P, E], f32, tag="bmb")
            # gate_w: one DMA into [p, (k e)] layout, then one bf16 copy.
            gw_in = persist.tile([P, KT, E], f32, tag="gw_in")
            nc.sync.dma_start(gw_in[:], gw_r[:])
            gw_bf = persist.tile([P, KT, E], bf16, tag="gw_bf")
            nc.vector.tensor_copy(gw_bf[:], gw_in[:])

            # PE warm-up while the input DMAs are in flight: keeps HAM busy
            # so the real matmuls start at 2.4 GHz instead of 1.2.
            wu_lhs = persist.tile([P, P], bf16, tag="wu_lhs")
            nc.vector.memset(wu_lhs[:], 0.0)
            wu_rhs = persist.tile([P, NH], bf16, tag="wu_rhs")
            nc.vector.memset(wu_rhs[:], 0.0)
            wu_ps = ps_gate.tile([P, NH], f32, tag="lg")
            for i in range(WARMUP_MMS):
                nc.tensor.matmul(
                    wu_ps[:], wu_lhs[:], wu_rhs[:],
                    start=(i == 0), stop=(i == WARMUP_MMS - 1),
                )

            xtb = []
            for k in range(KT):
                xf = p_wf.tile([P, TOK], f32, tag="xf")
                nc.sync.dma_start(xf[:], xT_r[k])
                xb = persist.tile([P, TOK], bf16, tag=f"xtb{k}")
                nc.vector.tensor_copy(xb[:], xf[:])
                xtb.append(xb)

            # ---- gating: logits -> softmax -> top-2 mask -> block mask ----
            cnt_ps = ps_cnt.tile([1, E], f32, tag="cnt")
            gfin = []
            for m in range(MT):
                lg = ps_gate.tile([P, E], f32, tag="lg")
                for k in range(KT):
                    nc.tensor.matmul(
                        lg[:], xtb[k][:, m * P:(m + 1) * P], gw_bf[:, k, :],
                        start=(k == 0), stop=False,
                    )
                # + gate_b (broadcast along tokens via K=1 matmul)
                nc.tensor.matmul(lg[:], ones_row_bf[:], gb_bf[:], start=False, stop=True)

                # softmax over the 8 experts (free dim). |logit| is O(1), so
                # no max-subtraction is needed for fp32 exp.
                ex = p_gs.tile([P, E], f32, tag="ex")
                nc.scalar.activation(ex[:], lg[:], AF.Exp)
                ssum = p_gs.tile([P, 1], f32, tag="ssum")
                nc.vector.reduce_sum(ssum[:], ex[:], axis=AX.X)
                rcp = p_gs.tile([P, 1], f32, tag="rcp")
                nc.vector.reciprocal(rcp[:], ssum[:])

                # top-2 mask: v >= (max of values with the argmax removed)
                m1 = p_gs.tile([P, 1], f32, tag="m1")
                nc.vector.reduce_max(m1[:], ex[:], axis=AX.X)
                eqb = p_gs.tile([P, E], f32, tag="eqb")
                nc.vector.tensor_scalar(
                    eqb[:], ex[:], m1[:], -1e30, op0=OP.is_ge, op1=OP.mult
                )
                g2 = p_gs.tile([P, E], f32, tag="g2")
                nc.vector.tensor_tensor(g2[:], ex[:], eqb[:], op=OP.add)
                m2 = p_gs.tile([P, 1], f32, tag="m2")
                nc.vector.reduce_max(m2[:], g2[:], axis=AX.X)
                msk = p_gs.tile([P, E], f32, tag="msk")
                nc.vector.tensor_scalar(msk[:], ex[:], m2[:], None, op0=OP.is_ge)
                # block-mask count: accumulate ones^T @ mask over all m tiles
                nc.tensor.matmul(
                    cnt_ps[:], ones_col[:], msk[:],
                    start=(m == 0), stop=(m == MT - 1),
                )

                gt = persist.tile([P, E], f32, tag=f"gt{m}")
                nc.vector.tensor_scalar_mul(gt[:], ex[:], rcp[:])
                gfin.append(gt)

            cnt_sb = p_gs.tile([1, E], f32, tag="cnt_sb")
            nc.vector.tensor_copy(cnt_sb[:], cnt_ps[:])
            bm01 = p_gs.tile([1, E], f32, tag="bm01")
            nc.vector.tensor_scalar(bm01[:], cnt_sb[:], 0.5, None, op0=OP.is_ge)
            # broadcast [1,E] -> [P,E] via K=1 matmul with a ones column
            bmb_ps = ps_gate.tile([P, E], f32, tag="bmb_ps", bufs=1)
            nc.tensor.matmul(bmb_ps[:], ones_row[:], bm01[:], start=True, stop=True)
            nc.vector.tensor_copy(bmb[:], bmb_ps[:])

            gsc = []
            for m in range(MT):
                gs2 = persist.tile([P, E], f32, tag=f"gsc{m}")
                nc.vector.tensor_tensor(gs2[:], gfin[m][:], bmb[:], op=OP.mult)
                gsc.append(gs2)

            # ---- main loop: out += gating[:, e] * (x @ W_e), expert-major ----
            acc = []
            for m in range(MT):
                a = persist.tile([P, D], f32, tag=f"acc{m}")
                acc.append(a)

            for e in range(E):
                wbf = []
                for k in range(KT):
                    wf = p_wf.tile([P, D], f32, tag="wf")
                    nc.sync.dma_start(wf[:], w_r[k, :, e, :])
                    wb = p_wb.tile([P, D], bf16, tag="wb")
                    nc.vector.tensor_copy(wb[:], wf[:])
                    wbf.append(wb)
                for m in range(MT):
                    gcol = gsc[m][:, e:e + 1]
                    ps0 = ps_mm.tile([P, NH], f32, tag="psmm")
                    ps1 = ps_mm.tile([P, NH], f32, tag="psmm")
                    for k in range(KT):
                        lhs = xtb[k][:, m * P:(m + 1) * P]
                        nc.tensor.matmul(
                            ps0[:], lhs, wbf[k][:, 0:NH],
                            start=(k == 0), stop=(k == KT - 1),
                        )
                        nc.tensor.matmul(
                            ps1[:], lhs, wbf[k][:, NH:D],
                            start=(k == 0), stop=(k == KT - 1),
                        )
                    for h, ps in ((0, ps0), (1, ps1)):
                        osl = acc[m][:, h * NH:(h + 1) * NH]
                        if e == 0:
                            nc.scalar.mul(osl, ps[:], gcol)
                        else:
                            tmp = p_tmp.tile([P, NH], f32, tag="tmp")
                            nc.scalar.mul(tmp[:], ps[:], gcol)
                            nc.vector.tensor_tensor(osl, osl, tmp[:], op=OP.add)

            for m in range(MT):
                nc.sync.dma_start(out_r[m], acc[m][:])

    nc.compile()
    return nc


_CACHE = {}
LAST_RESULTS = None  # BassKernelResults of the most recent run (for test.py)


def _get_nc():
    if "nc" not in _CACHE:
        _CACHE["nc"] = _build_nc()
    return _CACHE["nc"]


def kernel(x, W, gate_w, gate_b):
    global LAST_RESULTS
    from concourse.bass_utils import run_bass_kernel_spmd

    x = np.ascontiguousarray(np.asarray(x, dtype=np.float32))
    W = np.ascontiguousarray(np.asarray(W, dtype=np.float32))
    gate_w = np.ascontiguousarray(np.asarray(gate_w, dtype=np.float32))
    gb = np.ascontiguousarray(np.asarray(gate_b, dtype=np.float32).reshape(1, E))

    in_maps = []
    for c in range(N_CORES):
        xT = np.ascontiguousarray(x[c * TOK:(c + 1) * TOK].T)
        in_maps.append({"xT": xT, "w": W, "gate_w": gate_w, "gate_b": gb})

    res = run_bass_kernel_spmd(_get_nc(), in_maps, core_ids=list(range(N_CORES)))
    LAST_RESULTS = res
    return np.concatenate([r["out"] for r in res.results], axis=0)


# revision 15
# speedup vs baseline: 1.0101x; 1.0101x over previous
"""Trainium2 Bass kernel for nn_MoELayer (moe_routing).

Reference computation (B=8192 tokens, d=1024, E=8 experts, top-k=2):
    gating  = softmax(x @ gate_w + gate_b)                    # [B, E]
    mask    = top-2 one-hot scatter of gating                 # [B, E]
    blockm  = mask.reshape(B//d, d, E).max(axis=1)            # per 1024-row block
    out     = sum_e gating[:, e] * blockm[block(b), e] * (x @ W[:, e*d:(e+1)*d])

Key structural facts exploited here:
  * The combine uses the FULL softmax weights over all experts; the top-2
    mask enters only through the per-1024-row-block max. So the compute is
    dense: out = sum_e (gating*blockmask) .* (x @ W_e).
  * Both the gating and the block mask for a 1024-row block depend only on
    that block's rows.

Sharding: data-parallel over the 8 row blocks of 1024 tokens (one per
NeuronCore). Each core computes its whole output block locally; there is
no cross-core communication. W is streamed (replicated) to every core.

Per-core kernel:
  * x^T block (d on partitions) is DMA'd in fp32; gating logits are
    computed on the PE in fp32 (exact), softmax on ACT/DVE in fp32.
  * top-2 mask via (v >= second_max) per row; block mask via a
    ones-vector matmul partition-reduction; broadcast back with a K=1
    matmul.
  * The 8 [1024x1024x1024] expert matmuls run in bf16 (fp32 PSUM
    accumulation).  Per-expert combine: ACT copies PSUM->SBUF scaled by
    the per-token gating column (per-partition scale), DVE accumulates.
"""

import numpy as np

P = 128          # partitions
D = 1024         # d_model
E = 8            # experts
TOK = 1024       # tokens per core (row block)
KT = D // P      # contraction tiles
MT = TOK // P    # token tiles
NH = 512         # psum half-width (one fp32 bank)
N_CORES = 8
WARMUP_MMS = 42  # PE warm-up matmuls issued while input DMAs are in flight
BRIDGE_MMS = 20  # filler matmuls between gating and the main stream


def _build_nc():
    import concourse.bacc as bacc
    import concourse.mybir as mybir
    import concourse.tile as tile

    f32 = mybir.dt.float32
    bf16 = mybir.dt.bfloat16
    AX = mybir.AxisListType
    OP = mybir.AluOpType
    AF = mybir.ActivationFunctionType

    # Bacc (not raw Bass): its compile() pass splits excess per-instruction
    # semaphore waits into standalone event-semaphore instructions and moves
    # matmul waits onto LDWEIGHTS — required for walrus codegen's per-
    # instruction sync-wait limits.
    nc = bacc.Bacc(None, target_bir_lowering=False, debug=False)
    xT_d = nc.dram_tensor("xT", [D, TOK], f32, kind="ExternalInput")
    w_d = nc.dram_tensor("w", [D, E * D], f32, kind="ExternalInput")
    gw_d = nc.dram_tensor("gate_w", [D, E], f32, kind="ExternalInput")
    gb_d = nc.dram_tensor("gate_b", [1, E], f32, kind="ExternalInput")
    out_d = nc.dram_tensor("out", [TOK, D], f32, kind="ExternalOutput")

    xT_r = xT_d.rearrange("(k p) t -> k p t", p=P)
    w_r = w_d.rearrange("(k p) (e f) -> k p e f", p=P, f=D)
    gw_r = gw_d.rearrange("(k p) e -> p k e", p=P)
    out_r = out_d.rearrange("(m p) f -> m p f", p=P)

    with tile.TileContext(nc) as tc:
        with (
            tc.tile_pool(name="persist", bufs=1) as persist,
            tc.tile_pool(name="gstat", bufs=2) as p_gs,
            tc.tile_pool(name="wf", bufs=8) as p_wf,
            tc.tile_pool(name="wb", bufs=2 * KT) as p_wb,
            tc.tile_pool(name="tmp", bufs=6) as p_tmp,
            tc.tile_pool(name="ps_gate", bufs=2, space="PSUM") as ps_gate,
            tc.tile_pool(name="ps_cnt", bufs=1, space="PSUM") as ps_cnt,
            tc.tile_pool(name="ps_mm", bufs=4, space="PSUM") as ps_mm,
        ):
            # -- front matter: everything with no DMA dependency goes first so
            # the PE warm-up and the ACT exp-table load start at t~0.
            wu_lhs = persist.tile([P, P], bf16, tag="wu_lhs")
            nc.vector.memset(wu_lhs[:], 0.0)
            wu_rhs = persist.tile([P, NH], bf16, tag="wu_rhs")
            nc.vector.memset(wu_rhs[:], 0.0)
            ones_col = persist.tile([P, 1], f32, tag="ones_col")
            nc.vector.memset(ones_col[:], 1.0)
            ones_row = persist.tile([1, P], f32, tag="ones_row")
            nc.vector.memset(ones_row[:], 1.0)
            ones_row_bf = persist.tile([1, P], bf16, tag="ones_row_bf")
            nc.vector.memset(ones_row_bf[:], 1.0)
            # Preload the exp activation-table set (~2.7us) during DMA wait.
            exp_dummy = persist.tile([1, 1], f32, tag="exp_dummy")
            nc.scalar.activation(exp_dummy[:], ones_col[:1, :], AF.Exp)

            # PE warm-up while the input DMAs are in flight: keeps HAM busy
            # so the real matmuls start at 2.4 GHz instead of 1.2.
            wu_ps = ps_cnt.tile([P, NH], f32, tag="cnt")
            for i in range(WARMUP_MMS):
                nc.tensor.matmul(
                    wu_ps[:], wu_lhs[:], wu_rhs[:],
                    start=(i == 0), stop=(i == WARMUP_MMS - 1),
                )

            # gate_b / gate_w: DMA, then DVE copies so PE consumers wait on
            # the DVE semaphore, not a DMA queue (the LDWEIGHTS wait encoding
            # can't take two DMA-queue waits).
            gb_in = persist.tile([1, E], f32, tag="gb_in")
            nc.sync.dma_start(gb_in[:], gb_d[:])
            gb_bf = persist.tile([1, E], bf16, tag="gb_bf")
            nc.vector.tensor_copy(gb_bf[:], gb_in[:])
            bmb = persist.tile([P, E], f32, tag="bmb")
            # gate_w: one DMA into [p, (k e)] layout, then one bf16 copy.
            gw_in = persist.tile([P, KT, E], f32, tag="gw_in")
            nc.sync.dma_start(gw_in[:], gw_r[:])
            gw_bf = persist.tile([P, KT, E], bf16, tag="gw_bf")
            nc.vector.tensor_copy(gw_bf[:], gw_in[:])

            xtb = []
            for k in range(KT):
                xf = p_wf.tile([P, TOK], f32, tag="xf")
                nc.sync.dma_start(xf[:], xT_r[k])
                xb = persist.tile([P, TOK], bf16, tag=f"xtb{k}")
                nc.vector.tensor_copy(xb[:], xf[:])
                xtb.append(xb)

            # ---- gating: logits -> softmax -> top-2 mask -> block mask ----
            cnt_ps = ps_cnt.tile([1, E], f32, tag="cnt")
            gfin = []
            for m in range(MT):
                lg = ps_gate.tile([P, E], f32, tag="lg")
                for k in range(KT):
                    nc.tensor.matmul(
                        lg[:], xtb[k][:, m * P:(m + 1) * P], gw_bf[:, k, :],
                        start=(k == 0), stop=False,
                    )
                # + gate_b (broadcast along tokens via K=1 matmul)
                nc.tensor.matmul(lg[:], ones_row_bf[:], gb_bf[:], start=False, stop=True)

                # softmax over the 8 experts (free dim). |logit| is O(1), so
                # no max-subtraction is needed for fp32 exp.
                ex = p_gs.tile([P, E], f32, tag="ex")
                nc.scalar.activation(ex[:], lg[:], AF.Exp)
                ssum = p_gs.tile([P, 1], f32, tag="ssum")
                nc.vector.reduce_sum(ssum[:], ex[:], axis=AX.X)
                rcp = p_gs.tile([P, 1], f32, tag="rcp")
                nc.vector.reciprocal(rcp[:], ssum[:])

                # top-2 mask: v >= (max of values with the argmax removed)
                m1 = p_gs.tile([P, 1], f32, tag="m1")
                nc.vector.reduce_max(m1[:], ex[:], axis=AX.X)
                eqb = p_gs.tile([P, E], f32, tag="eqb")
                nc.vector.tensor_scalar(
                    eqb[:], ex[:], m1[:], -1e30, op0=OP.is_ge, op1=OP.mult
                )
                g2 = p_gs.tile([P, E], f32, tag="g2")
                nc.vector.tensor_tensor(g2[:], ex[:], eqb[:], op=OP.add)
                m2 = p_gs.tile([P, 1], f32, tag="m2")
                nc.vector.reduce_max(m2[:], g2[:], axis=AX.X)
                msk = p_gs.tile([P, E], f32, tag="msk")
                nc.vector.tensor_scalar(msk[:], ex[:], m2[:], None, op0=OP.is_ge)
                # block-mask count: accumulate ones^T @ mask over all m tiles
                nc.tensor.matmul(
                    cnt_ps[:], ones_col[:], msk[:],
                    start=(m == 0), stop=(m == MT - 1),
                )

                gt = persist.tile([P, E], f32, tag=f"gt{m}")
                nc.vector.tensor_scalar_mul(gt[:], ex[:], rcp[:])
                gfin.append(gt)

            cnt_sb = p_gs.tile([1, E], f32, tag="cnt_sb")
            nc.vector.tensor_copy(cnt_sb[:], cnt_ps[:])
            bm01 = p_gs.tile([1, E], f32, tag="bm01")
            nc.vector.tensor_scalar(bm01[:], cnt_sb[:], 0.5, None, op0=OP.is_ge)
            # broadcast [1,E] -> [P,E] via K=1 matmul with a ones column
            bmb_ps = ps_gate.tile([P, E], f32, tag="bmb_ps", bufs=1)
            nc.tensor.matmul(bmb_ps[:], ones_row[:], bm01[:], start=True, stop=True)
            nc.vector.tensor_copy(bmb[:], bmb_ps[:])

            gsc = []
            for m in range(MT):
                gs2 = persist.tile([P, E], f32, tag=f"gsc{m}")
                nc.vector.tensor_tensor(gs2[:], gfin[m][:], bmb[:], op=OP.mult)
                gsc.append(gs2)

            # Filler matmuls so the PE has no >3us idle window (which would
            # re-throttle HAM to 1.2 GHz) while the first expert's weights
            # finish streaming + converting.
            for i in range(BRIDGE_MMS):
                br_ps = ps_mm.tile([P, NH], f32, tag="psmm")
                nc.tensor.matmul(br_ps[:], wu_lhs[:], wu_rhs[:], start=True, stop=True)

            # ---- main loop: out += gating[:, e] * (x @ W_e), expert-major ----
            acc = []
            for m in range(MT):
                a = persist.tile([P, D], f32, tag=f"acc{m}")
                acc.append(a)

            for e in range(E):
                wbf = []
                for k in range(KT):
                    wf = p_wf.tile([P, D], f32, tag="wf")
                    nc.sync.dma_start(wf[:], w_r[k, :, e, :])
                    wb = p_wb.tile([P, D], bf16, tag="wb")
                    nc.vector.tensor_copy(wb[:], wf[:])
                    wbf.append(wb)
                for m in range(MT):
                    gcol = gsc[m][:, e:e + 1]
                    ps0 = ps_mm.tile([P, NH], f32, tag="psmm")
                    ps1 = ps_mm.tile([P, NH], f32, tag="psmm")
                    for k in range(KT):
                        lhs = xtb[k][:, m * P:(m + 1) * P]
                        nc.tensor.matmul(
                            ps0[:], lhs, wbf[k][:, 0:NH],
                            start=(k == 0), stop=(k == KT - 1),
                        )
                        nc.tensor.matmul(
                            ps1[:], lhs, wbf[k][:, NH:D],
                            start=(k == 0), stop=(k == KT - 1),
                        )
                    for h, ps in ((0, ps0), (1, ps1)):
                        osl = acc[m][:, h * NH:(h + 1) * NH]
                        if e == 0:
                            nc.scalar.mul(osl, ps[:], gcol)
                        else:
                            tmp = p_tmp.tile([P, NH], f32, tag="tmp")
                            nc.scalar.mul(tmp[:], ps[:], gcol)
                            nc.vector.tensor_tensor(osl, osl, tmp[:], op=OP.add)

            for m in range(MT):
                nc.sync.dma_start(out_r[m], acc[m][:])

    nc.compile()
    return nc


_CACHE = {}
LAST_RESULTS = None  # BassKernelResults of the most recent run (for test.py)


def _get_nc():
    if "nc" not in _CACHE:
        _CACHE["nc"] = _build_nc()
    return _CACHE["nc"]


def kernel(x, W, gate_w, gate_b):
    global LAST_RESULTS
    from concourse.bass_utils import run_bass_kernel_spmd

    x = np.ascontiguousarray(np.asarray(x, dtype=np.float32))
    W = np.ascontiguousarray(np.asarray(W, dtype=np.float32))
    gate_w = np.ascontiguousarray(np.asarray(gate_w, dtype=np.float32))
    gb = np.ascontiguousarray(np.asarray(gate_b, dtype=np.float32).reshape(1, E))

    in_maps = []
    for c in range(N_CORES):
        xT = np.ascontiguousarray(x[c * TOK:(c + 1) * TOK].T)
        in_maps.append({"xT": xT, "w": W, "gate_w": gate_w, "gate_b": gb})

    res = run_bass_kernel_spmd(_get_nc(), in_maps, core_ids=list(range(N_CORES)))
    LAST_RESULTS = res
    return np.concatenate([r["out"] for r in res.results], axis=0)


# revision 17
# speedup vs baseline: 1.0251x; 1.0148x over previous
"""Trainium2 Bass kernel for nn_MoELayer (moe_routing).

Reference computation (B=8192 tokens, d=1024, E=8 experts, top-k=2):
    gating  = softmax(x @ gate_w + gate_b)                    # [B, E]
    mask    = top-2 one-hot scatter of gating                 # [B, E]
    blockm  = mask.reshape(B//d, d, E).max(axis=1)            # per 1024-row block
    out     = sum_e gating[:, e] * blockm[block(b), e] * (x @ W[:, e*d:(e+1)*d])

Key structural facts exploited here:
  * The combine uses the FULL softmax weights over all experts; the top-2
    mask enters only through the per-1024-row-block max. So the compute is
    dense: out = sum_e (gating*blockmask) .* (x @ W_e).
  * Both the gating and the block mask for a 1024-row block depend only on
    that block's rows.

Sharding: data-parallel over the 8 row blocks of 1024 tokens (one per
NeuronCore). Each core computes its whole output block locally; there is
no cross-core communication. W is streamed (replicated) to every core.

Per-core schedule (PE is the bottleneck: 8x 1024^3 bf16 matmuls ~ 218us;
HBM streaming ~ 111us hides under it except the first ~8MB):
  * x^T and expert-0 weights are DMA'd interleaved; expert 0's matmuls are
    computed UNSCALED in two k-half passes into acc0, starting as soon as
    the first k-tiles arrive (no gating dependency). Its gating scale is
    folded in later (during experts 2-3) as acc += g0 * acc0.
  * Gating (logits matmuls, softmax, top-2 mask, block mask) runs between
    the two passes' tail and expert 1.
  * Experts 1..7: bf16 matmuls with fp32 PSUM accumulation over k; ACT
    applies the per-token gating column scaling PSUM->SBUF; DVE
    accumulates into acc.
  * A short PE warm-up keeps the HAM clock-gate at 2.4 GHz from the start.
"""

import numpy as np

P = 128          # partitions
D = 1024         # d_model
E = 8            # experts
TOK = 1024       # tokens per core (row block)
KT = D // P      # contraction tiles
KH = KT // 2     # k-half for expert 0's two passes
MT = TOK // P    # token tiles
NH = 512         # psum half-width (one fp32 bank)
N_CORES = 8
WARMUP_MMS = 10  # PE warm-up matmuls issued while the first DMAs land


def _build_nc():
    import concourse.bacc as bacc
    import concourse.mybir as mybir
    import concourse.tile as tile

    f32 = mybir.dt.float32
    bf16 = mybir.dt.bfloat16
    AX = mybir.AxisListType
    OP = mybir.AluOpType
    AF = mybir.ActivationFunctionType

    # Bacc (not raw Bass): its compile() pass splits excess per-instruction
    # semaphore waits into standalone event-semaphore instructions and moves
    # matmul waits onto LDWEIGHTS — required for walrus codegen's per-
    # instruction sync-wait limits.
    nc = bacc.Bacc(None, target_bir_lowering=False, debug=False)
    xT_d = nc.dram_tensor("xT", [D, TOK], f32, kind="ExternalInput")
    w_d = nc.dram_tensor("w", [D, E * D], f32, kind="ExternalInput")
    gw_d = nc.dram_tensor("gate_w", [D, E], f32, kind="ExternalInput")
    gb_d = nc.dram_tensor("gate_b", [1, E], f32, kind="ExternalInput")
    out_d = nc.dram_tensor("out", [TOK, D], f32, kind="ExternalOutput")

    xT_r = xT_d.rearrange("(k p) t -> k p t", p=P)
    w_r = w_d.rearrange("(k p) (e f) -> k p e f", p=P, f=D)
    gw_r = gw_d.rearrange("(k p) e -> p k e", p=P)
    out_r = out_d.rearrange("(m p) f -> m p f", p=P)

    with tile.TileContext(nc) as tc:
        with (
            tc.tile_pool(name="persist", bufs=1) as persist,
            tc.tile_pool(name="gstat", bufs=2) as p_gs,
            tc.tile_pool(name="wf", bufs=8) as p_wf,
            tc.tile_pool(name="wb", bufs=2 * KT) as p_wb,
            tc.tile_pool(name="tmp", bufs=6) as p_tmp,
            tc.tile_pool(name="ps_gate", bufs=2, space="PSUM") as ps_gate,
            tc.tile_pool(name="ps_cnt", bufs=1, space="PSUM") as ps_cnt,
            tc.tile_pool(name="ps_mm", bufs=4, space="PSUM") as ps_mm,
        ):
            # -- front matter: everything with no DMA dependency goes first so
            # the PE warm-up and the ACT exp-table load start at t~0.
            wu_lhs = persist.tile([P, P], bf16, tag="wu_lhs")
            nc.vector.memset(wu_lhs[:], 0.0)
            wu_rhs = persist.tile([P, NH], bf16, tag="wu_rhs")
            nc.vector.memset(wu_rhs[:], 0.0)
            ones_col = persist.tile([P, 1], f32, tag="ones_col")
            nc.vector.memset(ones_col[:], 1.0)
            ones_row = persist.tile([1, P], f32, tag="ones_row")
            nc.vector.memset(ones_row[:], 1.0)
            ones_row_bf = persist.tile([1, P], bf16, tag="ones_row_bf")
            nc.vector.memset(ones_row_bf[:], 1.0)
            # Preload the exp activation-table set (~2.7us) during DMA wait.
            exp_dummy = persist.tile([1, 1], f32, tag="exp_dummy")
            nc.scalar.activation(exp_dummy[:], ones_col[:1, :], AF.Exp)

            # PE warm-up while the first input DMAs are in flight: keeps HAM
            # busy so the real matmuls run at 2.4 GHz.
            wu_ps = ps_cnt.tile([P, NH], f32, tag="cnt")
            for i in range(WARMUP_MMS):
                nc.tensor.matmul(
                    wu_ps[:], wu_lhs[:], wu_rhs[:],
                    start=(i == 0), stop=(i == WARMUP_MMS - 1),
                )

            # -- loads: x^T k-tiles interleaved with expert-0 weight k-tiles
            # so expert 0's first matmuls can start as early as possible.
            # All PE operands are DVE-produced bf16 copies (PE then waits on
            # the DVE semaphore, never on two DMA queues — the LDWEIGHTS wait
            # encoding can't take two DMA-queue waits).
            xtb = []
            wbf0 = []
            for k in range(KT):
                xf = p_wf.tile([P, TOK], f32, tag="xf")
                nc.sync.dma_start(xf[:], xT_r[k])
                xb = persist.tile([P, TOK], bf16, tag=f"xtb{k}")
                nc.vector.tensor_copy(xb[:], xf[:])
                xtb.append(xb)
                wf = p_wf.tile([P, D], f32, tag="wf")
                nc.sync.dma_start(wf[:], w_r[k, :, 0, :])
                wb = p_wb.tile([P, D], bf16, tag="wb")
                nc.vector.tensor_copy(wb[:], wf[:])
                wbf0.append(wb)

            gb_in = persist.tile([1, E], f32, tag="gb_in")
            nc.sync.dma_start(gb_in[:], gb_d[:])
            gb_bf = persist.tile([1, E], bf16, tag="gb_bf")
            nc.vector.tensor_copy(gb_bf[:], gb_in[:])
            bmb = persist.tile([P, E], f32, tag="bmb")
            gw_in = persist.tile([P, KT, E], f32, tag="gw_in")
            nc.sync.dma_start(gw_in[:], gw_r[:])
            gw_bf = persist.tile([P, KT, E], bf16, tag="gw_bf")
            nc.vector.tensor_copy(gw_bf[:], gw_in[:])

            acc = []
            acc0 = []
            for m in range(MT):
                acc.append(persist.tile([P, D], f32, tag=f"acc{m}",
                                        name=f"acc{m}"))
                acc0.append(persist.tile([P, D], f32, tag=f"acc0{m}",
                                         name=f"acc0{m}"))

            # -- expert 0, pass A (k = 0..3), unscaled -> acc0
            for m in range(MT):
                ps0 = ps_mm.tile([P, NH], f32, tag="psmm")
                ps1 = ps_mm.tile([P, NH], f32, tag="psmm")
                for k in range(KH):
                    lhs = xtb[k][:, m * P:(m + 1) * P]
                    nc.tensor.matmul(ps0[:], lhs, wbf0[k][:, 0:NH],
                                     start=(k == 0), stop=(k == KH - 1))
                    nc.tensor.matmul(ps1[:], lhs, wbf0[k][:, NH:D],
                                     start=(k == 0), stop=(k == KH - 1))
                nc.scalar.copy(acc0[m][:, 0:NH], ps0[:])
                nc.scalar.copy(acc0[m][:, NH:D], ps1[:])

            # -- gating: logits -> softmax -> top-2 mask -> block mask
            cnt_ps = ps_cnt.tile([1, E], f32, tag="cnt")
            gfin = []
            for m in range(MT):
                lg = ps_gate.tile([P, E], f32, tag="lg")
                for k in range(KT):
                    nc.tensor.matmul(
                        lg[:], xtb[k][:, m * P:(m + 1) * P], gw_bf[:, k, :],
                        start=(k == 0), stop=False,
                    )
                # + gate_b (broadcast along tokens via K=1 matmul)
                nc.tensor.matmul(lg[:], ones_row_bf[:], gb_bf[:],
                                 start=False, stop=True)

                # softmax over the 8 experts (free dim). |logit| is O(1), so
                # no max-subtraction is needed for fp32 exp.
                ex = p_gs.tile([P, E], f32, tag="ex")
                nc.scalar.activation(ex[:], lg[:], AF.Exp)
                ssum = p_gs.tile([P, 1], f32, tag="ssum")
                nc.vector.reduce_sum(ssum[:], ex[:], axis=AX.X)
                rcp = p_gs.tile([P, 1], f32, tag="rcp")
                nc.vector.reciprocal(rcp[:], ssum[:])

                # top-2 mask: v >= (max of values with the argmax removed)
                m1 = p_gs.tile([P, 1], f32, tag="m1")
                nc.vector.reduce_max(m1[:], ex[:], axis=AX.X)
                eqb = p_gs.tile([P, E], f32, tag="eqb")
                nc.vector.tensor_scalar(
                    eqb[:], ex[:], m1[:], -1e30, op0=OP.is_ge, op1=OP.mult
                )
                g2 = p_gs.tile([P, E], f32, tag="g2")
                nc.vector.tensor_tensor(g2[:], ex[:], eqb[:], op=OP.add)
                m2 = p_gs.tile([P, 1], f32, tag="m2")
                nc.vector.reduce_max(m2[:], g2[:], axis=AX.X)
                msk = p_gs.tile([P, E], f32, tag="msk")
                nc.vector.tensor_scalar(msk[:], ex[:], m2[:], None, op0=OP.is_ge)
                # block-mask count: accumulate ones^T @ mask over all m tiles
                nc.tensor.matmul(
                    cnt_ps[:], ones_col[:], msk[:],
                    start=(m == 0), stop=(m == MT - 1),
                )

                gt = persist.tile([P, E], f32, tag=f"gt{m}")
                nc.vector.tensor_scalar_mul(gt[:], ex[:], rcp[:])
                gfin.append(gt)

            cnt_sb = p_gs.tile([1, E], f32, tag="cnt_sb")
            nc.vector.tensor_copy(cnt_sb[:], cnt_ps[:])
            bm01 = p_gs.tile([1, E], f32, tag="bm01")
            nc.vector.tensor_scalar(bm01[:], cnt_sb[:], 0.5, None, op0=OP.is_ge)
            # broadcast [1,E] -> [P,E] via K=1 matmul with a ones row
            bmb_ps = ps_gate.tile([P, E], f32, tag="bmb_ps", bufs=1)
            nc.tensor.matmul(bmb_ps[:], ones_row[:], bm01[:], start=True, stop=True)
            nc.vector.tensor_copy(bmb[:], bmb_ps[:])

            gsc = []
            for m in range(MT):
                gs2 = persist.tile([P, E], f32, tag=f"gsc{m}")
                nc.vector.tensor_tensor(gs2[:], gfin[m][:], bmb[:], op=OP.mult)
                gsc.append(gs2)

            # -- expert 0, pass B (k = 4..7), accumulate into acc0 on DVE
            for m in range(MT):
                ps0 = ps_mm.tile([P, NH], f32, tag="psmm")
                ps1 = ps_mm.tile([P, NH], f32, tag="psmm")
                for k in range(KH, KT):
                    lhs = xtb[k][:, m * P:(m + 1) * P]
                    nc.tensor.matmul(ps0[:], lhs, wbf0[k][:, 0:NH],
                                     start=(k == KH), stop=(k == KT - 1))
                    nc.tensor.matmul(ps1[:], lhs, wbf0[k][:, NH:D],
                                     start=(k == KH), stop=(k == KT - 1))
                nc.vector.tensor_tensor(acc0[m][:, 0:NH], acc0[m][:, 0:NH],
                                        ps0[:], op=OP.add)
                nc.vector.tensor_tensor(acc0[m][:, NH:D], acc0[m][:, NH:D],
                                        ps1[:], op=OP.add)

            # -- experts 1..7: acc (+)= g_e * (x @ W_e); expert 0's scaled
            # contribution g0 * acc0 is merged in during experts 2 and 3.
            for e in range(1, E):
                wbf = []
                for k in range(KT):
                    wf = p_wf.tile([P, D], f32, tag="wf")
                    nc.sync.dma_start(wf[:], w_r[k, :, e, :])
                    wb = p_wb.tile([P, D], bf16, tag="wb")
                    nc.vector.tensor_copy(wb[:], wf[:])
                    wbf.append(wb)
                for m in range(MT):
                    gcol = gsc[m][:, e:e + 1]
                    ps0 = ps_mm.tile([P, NH], f32, tag="psmm")
                    ps1 = ps_mm.tile([P, NH], f32, tag="psmm")
                    for k in range(KT):
                        lhs = xtb[k][:, m * P:(m + 1) * P]
                        nc.tensor.matmul(ps0[:], lhs, wbf[k][:, 0:NH],
                                         start=(k == 0), stop=(k == KT - 1))
                        nc.tensor.matmul(ps1[:], lhs, wbf[k][:, NH:D],
                                         start=(k == 0), stop=(k == KT - 1))
                    for h, ps in ((0, ps0), (1, ps1)):
                        osl = acc[m][:, h * NH:(h + 1) * NH]
                        if e == 1:
                            nc.scalar.mul(osl, ps[:], gcol)
                        else:
                            tmp = p_tmp.tile([P, NH], f32, tag="tmp")
                            nc.scalar.mul(tmp[:], ps[:], gcol)
                            nc.vector.tensor_tensor(osl, osl, tmp[:], op=OP.add)
                    if e in (2, 3):
                        # merge expert 0: acc += gsc[:,0] * acc0  (half the m
                        # tiles in e==2, the rest in e==3, to spread the load)
                        if (e == 2) == (m < MT // 2):
                            g0col = gsc[m][:, 0:1]
                            for h in range(2):
                                osl = acc[m][:, h * NH:(h + 1) * NH]
                                a0sl = acc0[m][:, h * NH:(h + 1) * NH]
                                tmp = p_tmp.tile([P, NH], f32, tag="tmp")
                                nc.scalar.mul(tmp[:], a0sl, g0col)
                                nc.vector.tensor_tensor(osl, osl, tmp[:],
                                                        op=OP.add)

            for m in range(MT):
                nc.sync.dma_start(out_r[m], acc[m][:])

    nc.compile()
    return nc


_CACHE = {}
LAST_RESULTS = None  # BassKernelResults of the most recent run (for test.py)


def _get_nc():
    if "nc" not in _CACHE:
        _CACHE["nc"] = _build_nc()
    return _CACHE["nc"]


def kernel(x, W, gate_w, gate_b):
    global LAST_RESULTS
    from concourse.bass_utils import run_bass_kernel_spmd

    x = np.ascontiguousarray(np.asarray(x, dtype=np.float32))
    W = np.ascontiguousarray(np.asarray(W, dtype=np.float32))
    gate_w = np.ascontiguousarray(np.asarray(gate_w, dtype=np.float32))
    gb = np.ascontiguousarray(np.asarray(gate_b, dtype=np.float32).reshape(1, E))

    in_maps = []
    for c in range(N_CORES):
        xT = np.ascontiguousarray(x[c * TOK:(c + 1) * TOK].T)
        in_maps.append({"xT": xT, "w": W, "gate_w": gate_w, "gate_b": gb})

    res = run_bass_kernel_spmd(_get_nc(), in_maps, core_ids=list(range(N_CORES)))
    LAST_RESULTS = res
    return np.concatenate([r["out"] for r in res.results], axis=0)
